# revision 16
# baseline (speedup 1.0000x reference)
"""Box-SDF (CAPUDF box boundary distance) Trainium2 Bass kernel.

For each 3-D point x (S = 0.4):
    q  = |x| - S
    d  = sqrt(sum_i relu(q_i)^2)    if any q_i >= 0   (outside)
    d  = -max_i q_i                 otherwise         (inside)

Branch-free formulation with the sqrt merged over both branches:
    a_i = |x_i|                      (uint16 view & 0x7FFF)
    b_i = max(a_i, S) - S            = relu(q_i)
    mx  = max_i a_i
    u   = min(mx, S) - S             (<= 0; = -inside-distance)
    d   = sqrt(b_0^2 + b_1^2 + b_2^2 + u^2)
(the b squares and u^2 are complementary: at most one side is nonzero,
so a single sqrt covers both the inside and outside branch.)

Perf structure (memory-regime problem; per-core HBM ~358 GB/s):
  - fp16 I/O: host converts inputs to planar fp16 and upconverts the
    fp16 output, halving HBM traffic (rel err ~4e-4 << 2e-2 gate).
  - abs via integer AND on a uint16 view (DVE tensor_scalar, 4x mode);
    b and u via fused two-scalar tensor_scalar ops (4x); max tree via
    two fp16 tensor_tensor ops (2x).
  - Squares: ACT Square for the 3 b planes, DVE for u (engine balance:
    ACT ~31us vs DVE ~31us per core); the last tile splits its squares
    across both engines to shorten the drain tail.
  - Plane sum via fp16 identity matmuls accumulating in PSUM (TensorE
    is otherwise idle), then ACT Sqrt PSUM->SBUF per 1024-col chunk.
  - Uneven tile widths [1024, 2048, 2048, 2048, 1024]: the small first
    tile (per-plane DMA+compute chunks) shortens the pipeline fill; the
    small last tile shortens the drain.
Sharding: data-parallel over the points axis across 8 NeuronCores.
"""

import sys

import numpy as np

sys.path.insert(0, "/opt/trn_rl_repo")

import concourse.bacc as bacc  # noqa: E402
import concourse.mybir as mybir  # noqa: E402
from concourse import bass_utils  # noqa: E402
from concourse.tile import TileContext  # noqa: E402

N = 8388608
NCORES = 8
NPC = N // NCORES  # 1,048,576 points per core
P = 128
CPP = NPC // P  # 8192 point-columns per partition per core
KS = [1024, 2048, 2048, 2048, 1024]  # tile widths (sum = CPP)
OFFS = [sum(KS[:i]) for i in range(len(KS))]
NT = len(KS)
C3 = 3 * CPP

SIZE = 0.4
F16 = mybir.dt.float16
F32 = mybir.dt.float32
U16 = mybir.dt.uint16
AF = mybir.ActivationFunctionType
OP = mybir.AluOpType
SIGN_MASK = 0x7FFF  # clears the fp16 sign bit


def build_kernel():
    nc = bacc.Bacc(
        "TRN2",
        target_bir_lowering=False,
        debug=False,
        num_devices=NCORES,
    )
    x = nc.dram_tensor("x", [P, C3], F16, kind="ExternalInput").ap()
    eye = nc.dram_tensor("eye", [P, P], F16, kind="ExternalInput").ap()
    d = nc.dram_tensor("d", [P, CPP], F16, kind="ExternalOutput").ap()

    with TileContext(nc) as tc:
        with (
            tc.tile_pool(name="const", bufs=1) as cpool,
            tc.tile_pool(name="xtp", bufs=3) as xtp,
            tc.tile_pool(name="apool", bufs=2) as apool,
            tc.tile_pool(name="bup", bufs=3) as bup,
            tc.tile_pool(name="sqp", bufs=3) as sqp,
            tc.tile_pool(name="small", bufs=2) as small,
            tc.tile_pool(name="dtp", bufs=3) as dtp,
            tc.tile_pool(name="psum", bufs=4, space="PSUM") as pspool,
        ):
            eye_t = cpool.tile([P, P], F16)
            state = {}

            def stage_a(t):
                k = KS[t]
                f3, f4 = 3 * k, 4 * k
                xt = xtp.tile([P, f3], F16, tag="xt")
                a = apool.tile([P, f3], F16, tag="a")
                bu = bup.tile([P, f4], F16, tag="bu")
                # Tile 0: small leading chunks so compute starts after ~128 KB.
                bounds = [0, 512, 1024, 2048, f3] if t == 0 else [0, f3]
                for c in range(len(bounds) - 1):
                    cs = slice(bounds[c], bounds[c + 1])
                    xs = slice(3 * OFFS[t] + bounds[c], 3 * OFFS[t] + bounds[c + 1])
                    nc.sync.dma_start(out=xt[:, cs], in_=x[:, xs])
                    if t == 0 and c == 0:
                        nc.sync.dma_start(out=eye_t[:], in_=eye[:])
                    # a = |x|: clear fp16 sign bits (DVE uint16 ts, 4x)
                    nc.vector.tensor_scalar(
                        out=a[:, cs].bitcast(U16),
                        in0=xt[:, cs].bitcast(U16),
                        scalar1=SIGN_MASK,
                        scalar2=None,
                        op0=OP.bitwise_and,
                    )
                    # b = max(a, S) - S  (DVE fp16 ts, 4x)
                    nc.vector.tensor_scalar(
                        out=bu[:, cs],
                        in0=a[:, cs],
                        scalar1=SIZE,
                        scalar2=-SIZE,
                        op0=OP.max,
                        op1=OP.add,
                    )

                # mx = max_i a_i (DVE fp16 max tree, 2x)
                m1 = small.tile([P, k], F16, tag="m1")
                nc.vector.tensor_tensor(
                    out=m1[:], in0=a[:, 0:k], in1=a[:, k : 2 * k], op=OP.max
                )
                mx = small.tile([P, k], F16, tag="mx")
                nc.vector.tensor_tensor(
                    out=mx[:], in0=m1[:], in1=a[:, 2 * k : 3 * k], op=OP.max
                )
                # u = min(mx, S) - S  (4th plane of bu, DVE ts 4x)
                nc.vector.tensor_scalar(
                    out=bu[:, f3:f4],
                    in0=mx[:],
                    scalar1=SIZE,
                    scalar2=-SIZE,
                    op0=OP.min,
                    op1=OP.add,
                )
                state[t] = bu

            def stage_b(t):
                k = KS[t]
                f3, f4 = 3 * k, 4 * k
                bu = state.pop(t)
                sq = sqp.tile([P, f4], F16, tag="sq")
                # Squares: ACT takes [0:sa], DVE [sa:f4].
                # Middle tiles: ACT does the 3 b planes, DVE just u.
                # Last tile: split roughly evenly so both engines drain fast.
                sa = 2 * k if t == NT - 1 else f3
                if t == 0:
                    for c in range(3):
                        nc.scalar.activation(
                            out=sq[:, c * k : (c + 1) * k],
                            in_=bu[:, c * k : (c + 1) * k],
                            func=AF.Square,
                        )
                else:
                    nc.scalar.activation(
                        out=sq[:, 0:sa], in_=bu[:, 0:sa], func=AF.Square
                    )
                nc.vector.tensor_tensor(
                    out=sq[:, sa:f4],
                    in0=bu[:, sa:f4],
                    in1=bu[:, sa:f4],
                    op=OP.mult,
                )

                # s = sum of 4 planes via fp16 identity matmuls into PSUM,
                # in 1024-col chunks so sqrt/out can start early.
                hw = 1024
                for h in range(0, k, hw):
                    s_ps = pspool.tile([P, hw], F32, tag="s_ps")
                    for j in range(h, h + hw, 512):
                        for c in range(4):
                            nc.tensor.matmul(
                                s_ps[:, j - h : j - h + 512],
                                eye_t[:],
                                sq[:, c * k + j : c * k + j + 512],
                                start=(c == 0),
                                stop=(c == 3),
                            )
                    dt_ = dtp.tile([P, hw], F16, tag="dt")
                    nc.scalar.activation(out=dt_[:], in_=s_ps[:], func=AF.Sqrt)
                    nc.sync.dma_start(
                        out=d[:, OFFS[t] + h : OFFS[t] + h + hw], in_=dt_[:]
                    )

            # 2-stage software pipeline emission: A(t+1) before B(t) so each
            # engine's in-order stream never stalls tile t+1's front work
            # behind tile t's tail work.
            stage_a(0)
            for t in range(1, NT):
                stage_a(t)
                stage_b(t - 1)
            stage_b(NT - 1)

    nc.compile()
    return nc


_cached_nc = None


def _get_nc():
    global _cached_nc
    if _cached_nc is None:
        _cached_nc = build_kernel()
    return _cached_nc


_AXON_SO = "/opt/axon/libaxon_pjrt.so"


def _ensure_ntff_hook():
    """Install an antenv.axon_hooks shim backed by libaxon_pjrt's NRT
    profiling C ABI, so run_bass_kernel_spmd(trace=True) works under axon."""
    try:
        from antenv.axon_hooks import get_axon_ntff_profile_hook  # noqa: F401

        return
    except ImportError:
        pass
    import contextlib
    import ctypes
    import types

    import antenv

    holder = {}
    mod = types.ModuleType("antenv.axon_hooks")
    mod.set_axon_ntff_profile_hook = lambda h: holder.__setitem__("h", h)
    mod.get_axon_ntff_profile_hook = lambda: holder.get("h")
    sys.modules["antenv.axon_hooks"] = mod
    antenv.axon_hooks = mod

    try:
        lib = ctypes.CDLL(_AXON_SO)
    except OSError:
        return
    if not hasattr(lib, "axon_start_nrt_profile"):
        return
    lib.axon_start_nrt_profile.argtypes = [
        ctypes.POINTER(ctypes.c_int64),
        ctypes.c_size_t,
    ]
    lib.axon_start_nrt_profile.restype = ctypes.c_int64
    lib.axon_stop_nrt_profile.argtypes = [ctypes.c_char_p]
    lib.axon_stop_nrt_profile.restype = ctypes.c_int64

    @contextlib.contextmanager
    def _hook(output_dir, device_ids):
        import jax

        jax.devices()
        if device_ids:
            ids = (ctypes.c_int64 * len(device_ids))(*device_ids)
            rc = lib.axon_start_nrt_profile(ids, len(device_ids))
        else:
            rc = lib.axon_start_nrt_profile(None, 0)
        if rc != 0:
            raise RuntimeError(f"axon_start_nrt_profile rc={rc}")
        try:
            yield
        finally:
            n = lib.axon_stop_nrt_profile(str(output_dir).encode())
            print(f"ntff profile: {n} file(s) written to {output_dir}")

    holder["h"] = _hook


def _pack_inputs(pts):
    """[N,3] fp32 -> per-core planar fp16 [NCORES, P, C3] with per-tile
    planar blocks at column offsets 3*OFFS[t]."""
    h = pts.astype(np.float16).reshape(NCORES, P, CPP, 3)
    shards = np.empty((NCORES, P, C3), dtype=np.float16)
    for t, k in enumerate(KS):
        blk = h[:, :, OFFS[t] : OFFS[t] + k, :].transpose(0, 1, 3, 2)
        shards[:, :, 3 * OFFS[t] : 3 * OFFS[t] + 3 * k] = blk.reshape(
            NCORES, P, 3 * k
        )
    return shards


def run(inputs_array, trace=False, **kwargs):
    """inputs_array: [N, 3] float32. Returns (out [N] float32, BassKernelResults)."""
    pts = np.ascontiguousarray(inputs_array, dtype=np.float32)
    assert pts.shape == (N, 3), pts.shape
    shards = _pack_inputs(pts)
    if trace:
        _ensure_ntff_hook()
    nc = _get_nc()
    eye_np = np.eye(P, dtype=np.float16)
    in_maps = [{"x": shards[i], "eye": eye_np} for i in range(NCORES)]
    res = bass_utils.run_bass_kernel_spmd(
        nc, in_maps, core_ids=list(range(NCORES)), trace=trace, **kwargs
    )
    out = np.concatenate(
        [res.results[i]["d"].reshape(-1) for i in range(NCORES)]
    ).astype(np.float32)
    return out, res


def kernel(**inputs):
    out, _ = run(inputs["inputs"])
    return out


if __name__ == "__main__":
    rng = np.random.default_rng(0)
    pts = rng.standard_normal((N, 3)).astype(np.float32)
    out, _ = run(pts)
    q = np.abs(pts) - SIZE
    inside = np.all(q < 0, axis=1)
    d_out = np.sqrt(np.sum(np.square(np.maximum(q, 0.0)), axis=1))
    d_in = -np.max(q, axis=1)
    exp = np.where(inside, d_in, d_out)
    err = np.abs(out - exp) / np.maximum(np.abs(exp), 1e-6)
    print("max rel err:", err.max(), "mean:", err.mean())
